# revision 17
# baseline (speedup 1.0000x reference)
"""BinConv2d (XNOR-Net style) Trainium2 kernel, 8-core data-parallel, v3.

Layer math (BatchNorm train-mode -> BinActiv -> binary 3x3 conv -> scale by
box-filtered channel-mean magnitudes and per-filter alpha -> relu):

  mu, var: batch stats of x over (N, H, W) per channel      (needs all-reduce)
  xn  = (x - mu) * rsqrt(var + eps) * gamma + beta
  m   = mean_c |xn|;  xb = sign(xn);  Wb = sign(W);  alpha = mean |W| per filter
  y   = conv(xb, Wb, pad=1) + b
  out = relu(y * box3x3(m) * alpha)

Structure (the stats AllGather has ~70-120us ncfw setup latency; everything
that does not depend on the *global* stats runs while it is in flight):

  pass 1:  load x (bf16, host-converted) into resident SBUF, one bn_stats per
           image-half, trigger the AllGather as early as possible (~40us).
  window:  weight prep (PE transpose + ACT sign -> fp8, alpha via ACT
           abs-accumulate), and the m/beta path computed with LOCAL per-core
           BN stats (the m path is a smooth functional of the stats; local
           stats perturb it ~1e-3 relative, far under the harness gate):
           ACT abs -> fp8 |xn|, one DoubleRow matmul per row-chunk on a
           dedicated PSUM bank, DVE copies into the padded m plane, box3x3
           via DVE row-sums + banded matmul, then gpsimd partition_broadcast
           into abeta[128, img, pix] (bf16).
  pass 2:  global scalar chain -> t'; ACT sign (exact global stats) -> fp8
           conv via 9 shifted DoubleRow matmuls (PSUM 4+3 banks per image
           half, conv-exclusive) -> fused relu(scale,bias) -> multiply by
           abeta -> bf16 DMA out.

  sign needs *global* mu (a local-stats threshold flips ~0.24% of pixels =>
  ~10% output error), so the conv cannot start before the collective lands;
  the m path tolerates local stats.
"""

import os
import sys

import numpy as np

for _p in ("/opt/trn_rl_repo", "/root/.axon_site/_ro/trn_rl_repo"):
    if os.path.isdir(_p) and _p not in sys.path:
        sys.path.insert(0, _p)

import concourse.bass as bass  # noqa: E402
import concourse.bacc as bacc  # noqa: E402
import concourse.mybir as mybir  # noqa: E402
import concourse.tile as tile  # noqa: E402
from concourse.bass_utils import run_bass_kernel_spmd  # noqa: E402

F32 = mybir.dt.float32
BF16 = mybir.dt.bfloat16
FP8 = mybir.dt.float8e4
NPBF16 = mybir.dt.np(BF16)
AF = mybir.ActivationFunctionType
ALU = mybir.AluOpType
AX = mybir.AxisListType

EPS = 1e-4
NCORES = 8
P = 128
CIN = 256
COUT = 256
H = 56
W = 56
HP = H + 2          # 58 padded rows
WP = W + 2          # 58 padded cols
IMGP = HP * WP      # 3364 padded pixels / image
NPIX = H * W        # 3136 true pixels / image
MARGIN = 64         # dead zero margin absorbing out-of-image tap reads
CH_ROWS = 8         # output rows per PSUM bank
NCH = H // CH_ROWS  # 7 chunks / image
CF = CH_ROWS * W    # 448 compact free elems / chunk
CFP = CH_ROWS * WP  # 464 padded free elems / chunk
BANK = 512          # f32 elems per PSUM bank
KTAPS = 9
WARMUP = os.environ.get("BC_WARMUP", "1") == "1"


def _build(n_local: int):
    NL = n_local
    FREEPAD = 2 * MARGIN + NL * IMGP

    nc = bacc.Bacc("TRN2", debug=False, target_bir_lowering=False,
                   num_devices=NCORES)
    x_d = nc.declare_dram_parameter("x", [NL, CIN, H, W], BF16, isOutput=False)
    g_d = nc.declare_dram_parameter("gamma", [CIN], F32, isOutput=False)
    bb_d = nc.declare_dram_parameter("beta_bn", [CIN], F32, isOutput=False)
    w_d = nc.declare_dram_parameter("W", [COUT, CIN, 3, 3], BF16, isOutput=False)
    b_d = nc.declare_dram_parameter("b", [COUT], F32, isOutput=False)
    id_d = nc.declare_dram_parameter("ident", [P, P], BF16, isOutput=False)
    tv_d = nc.declare_dram_parameter("tvt", [HP, H], BF16, isOutput=False)
    out_d = nc.declare_dram_parameter("out", [NL, COUT, H, W], BF16, isOutput=True)

    with tile.TileContext(nc, num_cores=NCORES) as tc:
        with (
            tc.tile_pool(name="statics", bufs=1) as st,
            tc.tile_pool(name="wn", bufs=2) as wn_p,
            tc.tile_pool(name="axp", bufs=3) as axp,
            tc.tile_pool(name="smalls", bufs=2) as sm,
            tc.tile_pool(name="zp", bufs=2) as zp,
            tc.tile_pool(name="outp", bufs=3) as outp,
            tc.tile_pool(name="psA", bufs=1, space="PSUM") as psA,
            tc.tile_pool(name="psB", bufs=1, space="PSUM") as psB,
            tc.tile_pool(name="psS", bufs=1, space="PSUM") as psS,
            tc.tile_pool(name="dram", bufs=1, space="DRAM") as dr,
        ):
            # ---------------- warmup collective ----------------
            # ncfw pays ~70-120us of one-time setup on the first collective;
            # fire a throwaway AllGather with no data deps immediately so the
            # real one only pays mesh latency.
            if WARMUP:
                wu_in = dr.tile([1, 8], F32, name="wu_in", tag="wu_in")
                wu_out = dr.tile([NCORES, 1, 8], F32, name="wu_out",
                                 tag="wu_out", addr_space="Shared")
                nc.gpsimd.collective_compute(
                    "AllGather", ALU.bypass,
                    replica_groups=[list(range(NCORES))],
                    ins=[wu_in.opt()], outs=[wu_out.opt()],
                )

            # ---------------- pass 1: load x + BN statistics ----------------
            # (emitted first so the collective trigger lands ~40us in)
            xr = st.tile([P, 2 * NL * NPIX], BF16, name="xr", tag="xr")
            xrv = xr.rearrange("p (k i f) -> p k i f", k=2, i=NL)
            # DVE does bn_stats for kc0 (all imgs) + kc1 img0; ACT accumulates
            # raw sum/sumsq for kc1 imgs 1..3 (a pure-DVE pass-1 serializes 56
            # bn_stats and delays the collective trigger by ~25us).
            ACT_STATS = [(1, i) for i in range(1, NL)]
            stats = []
            for kc in range(2):
                ngrp = NCH * (NL if kc == 0 else 1)
                sb = st.tile([P, ngrp * 6], F32, name=f"stats{kc}",
                             tag=f"stats{kc}")
                stats.append(sb)
            acc_s, acc_q = {}, {}
            trash = st.tile([P, NPIX], BF16, name="trash", tag="trash")
            for img in range(NL):
                for kc in range(2):
                    deng = nc.sync if kc == 0 else nc.scalar
                    deng.dma_start(
                        xrv[:, kc, img, :],
                        x_d.ap()[img, kc * P:(kc + 1) * P]
                        .rearrange("c h w -> c (h w)"),
                    )
                    if (kc, img) in ACT_STATS:
                        s_t = sm.tile([P, 1], F32, name="s_t", tag=f"s_t{img}")
                        q_t = sm.tile([P, 1], F32, name="q_t", tag=f"q_t{img}")
                        nc.scalar.activation(trash[:], xrv[:, kc, img, :],
                                             AF.Copy, accum_out=s_t[:])
                        nc.scalar.activation(trash[:], xrv[:, kc, img, :],
                                             AF.Square, accum_out=q_t[:])
                        acc_s[(kc, img)] = s_t
                        acc_q[(kc, img)] = q_t
                    else:
                        ioff = img if kc == 0 else 0
                        for g in range(NCH):
                            col = (ioff * NCH + g) * 6
                            nc.vector.bn_stats(
                                stats[kc][:, col:col + 6],
                                xrv[:, kc, img, g * CF:(g + 1) * CF],
                            )
            # per-half (mean, ex2, var): kc0 pure bn_aggr; kc1 combines the
            # img0 aggregate with the ACT raw sums (equal image weights)
            NHALF = NL * NPIX
            mean_h, ex2_h, var_h = [], [], []
            pay = st.tile([P, 4], F32, name="pay", tag="pay")
            a0 = st.tile([P, 2], F32, name="a0", tag="a0")
            nc.vector.bn_aggr(a0[:], stats[0][:])
            m0sq = sm.tile([P, 1], F32, name="m0sq", tag="m0sq")
            nc.vector.tensor_mul(m0sq[:], a0[:, 0:1], a0[:, 0:1])
            ex0 = st.tile([P, 1], F32, name="ex0", tag="ex0")
            nc.vector.tensor_add(ex0[:], a0[:, 1:2], m0sq[:])
            nc.vector.tensor_copy(pay[:, 0:1], a0[:, 0:1])
            nc.vector.tensor_copy(pay[:, 1:2], ex0[:])
            mean_h.append(a0[:, 0:1]); ex2_h.append(ex0[:])
            var_h.append(a0[:, 1:2])
            a1 = st.tile([P, 2], F32, name="a1", tag="a1")
            nc.vector.bn_aggr(a1[:], stats[1][:])
            # S = NPIX*mean_img0 + sum(s_i); Q = NPIX*(var+mean^2) + sum(q_i)
            Ssum = sm.tile([P, 1], F32, name="Ssum", tag="Ssum")
            nc.vector.tensor_add(Ssum[:], acc_s[(1, 1)][:], acc_s[(1, 2)][:])
            nc.vector.tensor_add(Ssum[:], Ssum[:], acc_s[(1, 3)][:])
            m1n = sm.tile([P, 1], F32, name="m1n", tag="m1n")
            nc.vector.tensor_scalar_mul(m1n[:], a1[:, 0:1], float(NPIX))
            nc.vector.tensor_add(Ssum[:], Ssum[:], m1n[:])
            Qsum = sm.tile([P, 1], F32, name="Qsum", tag="Qsum")
            nc.vector.tensor_add(Qsum[:], acc_q[(1, 1)][:], acc_q[(1, 2)][:])
            nc.vector.tensor_add(Qsum[:], Qsum[:], acc_q[(1, 3)][:])
            m1sq = sm.tile([P, 1], F32, name="m1sq", tag="m1sq")
            nc.vector.tensor_mul(m1sq[:], a1[:, 0:1], a1[:, 0:1])
            e1 = sm.tile([P, 1], F32, name="e1", tag="e1")
            nc.vector.tensor_add(e1[:], a1[:, 1:2], m1sq[:])
            e1n = sm.tile([P, 1], F32, name="e1n", tag="e1n")
            nc.vector.tensor_scalar_mul(e1n[:], e1[:], float(NPIX))
            nc.vector.tensor_add(Qsum[:], Qsum[:], e1n[:])
            mean1 = st.tile([P, 1], F32, name="mean1", tag="mean1")
            nc.vector.tensor_scalar_mul(mean1[:], Ssum[:], 1.0 / NHALF)
            ex21 = st.tile([P, 1], F32, name="ex21", tag="ex21")
            nc.vector.tensor_scalar_mul(ex21[:], Qsum[:], 1.0 / NHALF)
            nc.vector.tensor_copy(pay[:, 2:3], mean1[:])
            nc.vector.tensor_copy(pay[:, 3:4], ex21[:])
            mn1sq = sm.tile([P, 1], F32, name="mn1sq", tag="mn1sq")
            nc.vector.tensor_mul(mn1sq[:], mean1[:], mean1[:])
            var1 = st.tile([P, 1], F32, name="var1", tag="var1")
            nc.vector.tensor_sub(var1[:], ex21[:], mn1sq[:])
            mean_h.append(mean1[:]); ex2_h.append(ex21[:])
            var_h.append(var1[:])
            cc_in = dr.tile([P, 4], F32, name="cc_in", tag="cc_in")
            cc_out = dr.tile([NCORES, P, 4], F32, name="cc_out", tag="cc_out",
                             addr_space="Shared")
            nc.sync.dma_start(cc_in[:], pay[:])
            nc.gpsimd.collective_compute(
                "AllGather", ALU.bypass,
                replica_groups=[list(range(NCORES))],
                ins=[cc_in.opt()], outs=[cc_out.opt()],
            )

            # ---------------- static pads (gpsimd; DVE stays on stats) ------
            # xq blocks: [P, img, ko, M | IMGP | M] so the DoubleRow rhs view
            # (k-stride = BLK) bounding-boxes only ONE image's two halves --
            # a [P, 2*plane] layout makes conv(img) falsely depend on later
            # images' sign writes (subtile deps use bounding ranges).
            BLK = IMGP + 2 * MARGIN
            xq = st.tile([P, NL * 2 * BLK], FP8, name="xq", tag="xq")
            xqb = xq.rearrange("p (i k b) -> p i k b", i=NL, k=2)
            for img in range(NL):
                for ko in range(2):
                    nc.gpsimd.memset(xqb[:, img, ko, 0:MARGIN + WP], 0.0)
                    nc.gpsimd.memset(
                        xqb[:, img, ko, MARGIN + (HP - 1) * WP:BLK], 0.0)
                    colv = (xqb[:, img, ko, MARGIN + WP: MARGIN + (HP - 1) * WP]
                            .rearrange("p (h w) -> p h w", w=WP))
                    nc.gpsimd.memset(colv[:, :, 0:1], 0.0)
                    nc.gpsimd.memset(colv[:, :, WP - 1:WP], 0.0)
            m_flat = st.tile([1, NL * IMGP], BF16, name="m_flat", tag="m_flat")
            mfl = m_flat.rearrange("p (i f) -> p i f", i=NL)
            nc.gpsimd.memset(mfl[:, :, 0:WP], 0.0)
            nc.gpsimd.memset(mfl[:, :, (HP - 1) * WP:IMGP], 0.0)
            mfv = mfl[:, :, WP:(HP - 1) * WP].rearrange("p i (h w) -> p i h w",
                                                        w=WP)
            nc.gpsimd.memset(mfv[:, :, :, 0:1], 0.0)
            nc.gpsimd.memset(mfv[:, :, :, WP - 1:WP], 0.0)
            epsc = st.tile([P, 1], F32, name="epsc", tag="epsc")
            nc.vector.memset(epsc[:], EPS)
            ones2 = st.tile([P, 2], FP8, name="ones2", tag="ones2")
            nc.vector.memset(ones2[:], 1.0)

            # ---------------- host constants ----------------
            ident = st.tile([P, P], BF16, name="ident_sb", tag="ident_sb")
            nc.scalar.dma_start(ident[:], id_d.ap())
            tvt = st.tile([HP, H], BF16, name="tvt_sb", tag="tvt_sb")
            nc.scalar.dma_start(tvt[:], tv_d.ap())
            gam2 = st.tile([P, 2], F32, name="gam2", tag="gam2")
            nc.scalar.dma_start(gam2[:], g_d.ap().rearrange("(k p) -> p k", k=2))
            bet2 = st.tile([P, 2], F32, name="bet2", tag="bet2")
            nc.scalar.dma_start(bet2[:], bb_d.ap().rearrange("(k p) -> p k", k=2))
            bvec2 = st.tile([P, 2], F32, name="bvec2", tag="bvec2")
            nc.scalar.dma_start(bvec2[:], b_d.ap().rearrange("(k p) -> p k", k=2))

            # ---------------- window: local-stat scalars ----------------
            s_loc, bstar = [], []
            for kc in range(2):
                sigl = sm.tile([P, 1], F32, name="sigl", tag="sigl")
                nc.scalar.activation(sigl[:], var_h[kc], AF.Sqrt,
                                     bias=epsc[:])
                rsigl = sm.tile([P, 1], F32, name="rsigl", tag="rsigl")
                nc.vector.reciprocal(rsigl[:], sigl[:])
                sl = st.tile([P, 1], F32, name=f"sl{kc}", tag=f"sl{kc}")
                nc.vector.tensor_mul(sl[:], gam2[:, kc:kc + 1], rsigl[:])
                s_loc.append(sl)
                smu = sm.tile([P, 1], F32, name="smu", tag="smu")
                nc.vector.tensor_mul(smu[:], sl[:], mean_h[kc])
                bs = st.tile([P, 1], F32, name=f"bs{kc}", tag=f"bs{kc}")
                nc.vector.tensor_sub(bs[:], bet2[:, kc:kc + 1], smu[:])
                bstar.append(bs)

            # ---------------- window: weight prep ----------------
            w_nat = []
            for oc in range(2):
                wn = wn_p.tile([P, CIN * KTAPS], BF16, name="w_nat", tag="wn")
                nc.sync.dma_start(
                    wn[:],
                    w_d.ap()[oc * P:(oc + 1) * P]
                    .rearrange("o c kh kw -> o (c kh kw)"),
                )
                w_nat.append(wn)
            # alpha via ACT |.| accumulate (tensor_reduce on DVE would race the
            # stats/pay chain and delay the collective trigger)
            alpha_sc, ab = [], []
            for oc in range(2):
                araw = sm.tile([P, 1], F32, name="araw", tag="araw")
                nc.scalar.activation(trash[:, 0:CIN * KTAPS], w_nat[oc][:],
                                     AF.Abs, accum_out=araw[:])
                asc = st.tile([P, 1], F32, name=f"alph{oc}", tag=f"alph{oc}")
                nc.vector.tensor_scalar_mul(asc[:], araw[:], 1.0 / (CIN * KTAPS))
                alpha_sc.append(asc)
                abt = st.tile([P, 1], F32, name=f"ab{oc}", tag=f"ab{oc}")
                nc.vector.tensor_mul(abt[:], asc[:], bvec2[:, oc:oc + 1])
                ab.append(abt)
            # wq: sign(W) transposed into DoubleRow lhsT layout
            # [P(ki), tap, oc, ko, m] with ko = channel half (c = ko*128+ki)
            wq = st.tile([P, KTAPS * 2 * 2 * P], FP8, name="wq", tag="wq")
            wqv = wq.rearrange("p (t o k m) -> p t o k m", t=KTAPS, o=2, k=2)
            for oc in range(2):
                wv = w_nat[oc][:].rearrange("o (c t) -> o c t", t=KTAPS)
                for kcw in range(2):
                    for tap in range(KTAPS):
                        pool, tag = ((psA, "cvA") if (tap + kcw) % 2 == 0
                                     else (psB, "cvB"))
                        psT = pool.tile([P, P], BF16, name="psT", tag=tag)
                        nc.tensor.transpose(
                            psT[:], wv[:, kcw * P:(kcw + 1) * P, tap], ident[:])
                        nc.scalar.activation(wqv[:, tap, oc, kcw, :], psT[:],
                                             AF.Sign)

            # ---------------- window: m path with LOCAL stats ----------------
            # ax = |s_loc*x + bstar_loc| = |xn_loc| in fp8 (3% elem error
            # averages to ~0.1% on m), one DoubleRow matmul per chunk against
            # all-ones [P,2,1], 1/(CIN*9) folded into tvt.
            abeta = st.tile([P, NL * NPIX], BF16, name="abeta", tag="abeta")
            abv = abeta.rearrange("p (i f) -> p i f", i=NL)
            for img in range(NL):
                ax = axp.tile([P, 2 * NPIX], FP8, name="ax", tag="ax")
                axv = ax.rearrange("p (k f) -> p k f", k=2)
                for kc in range(2):
                    nc.scalar.activation(axv[:, kc, :], xrv[:, kc, img, :],
                                         AF.Abs, bias=bstar[kc][:],
                                         scale=s_loc[kc][:])
                for ch in range(NCH):
                    mp = psS.tile([1, CF], F32, name="mp", tag="s")
                    nc.tensor.matmul(mp[:], ones2[:, 0:1],
                                     axv[:, 0, ch * CF:(ch + 1) * CF],
                                     start=True, stop=False)
                    nc.tensor.matmul(mp[:], ones2[:, 1:2],
                                     axv[:, 1, ch * CF:(ch + 1) * CF],
                                     start=False, stop=True)
                    mfi = (m_flat[:, img * IMGP:(img + 1) * IMGP]
                           .rearrange("p (h w) -> p h w", w=WP))
                    nc.vector.tensor_copy(
                        mfi[:, 1 + ch * CH_ROWS: 1 + (ch + 1) * CH_ROWS,
                            1:1 + W],
                        mp.rearrange("p (h w) -> p h w", w=W),
                    )
                # beta map: horizontal sum on DVE, vertical via banded matmul
                mhw = sm.tile([HP, WP], BF16, name="mhw", tag="mhw")
                nc.sync.dma_start(mhw[:], m_flat[:, img * IMGP:(img + 1) * IMGP])
                hs = sm.tile([HP, WP], BF16, name="hs", tag="hs")
                nc.vector.tensor_add(hs[:, 1:1 + W], mhw[:, 0:W], mhw[:, 2:2 + W])
                nc.vector.tensor_add(hs[:, 1:1 + W], hs[:, 1:1 + W],
                                     mhw[:, 1:1 + W])
                bps = psS.tile([H, W], F32, name="bps", tag="s")
                nc.tensor.matmul(bps[:], tvt[:], hs[:, 1:1 + W], start=True,
                                 stop=True)
                bhw = sm.tile([H, W], BF16, name="bhw", tag="bhw")
                nc.vector.tensor_copy(bhw[:], bps[:])
                bflat = sm.tile([1, NPIX], BF16, name="bflat", tag="bflat",
                                bufs=2)
                nc.sync.dma_start(bflat[:], bhw[:])
                nc.gpsimd.partition_broadcast(abv[:, img, :], bflat[:])

            # ---------------- global stats readback + scalars ----------------
            ag_sb = st.tile([P, NCORES * 4], F32, name="ag_sb", tag="ag_sb")
            nc.sync.dma_start(
                ag_sb[:].rearrange("p (r c) -> p r c", c=4),
                cc_out.rearrange("r p c -> p r c"),
            )
            arsb = st.tile([P, 4], F32, name="arsb", tag="arsb")
            nc.vector.tensor_reduce(
                arsb[:], ag_sb[:].rearrange("p (r c) -> p c r", c=4),
                axis=AX.X, op=ALU.add,
            )
            arv = arsb.rearrange("p (c k) -> p c k", c=2)  # [P, kc, (mean,ex2)]
            muv = st.tile([P, 2], F32, name="muv", tag="muv")
            nc.vector.tensor_scalar_mul(muv[:], arv[:, :, 0], 1.0 / NCORES)
            ex2v = sm.tile([P, 2], F32, name="ex2v", tag="ex2v")
            nc.vector.tensor_scalar_mul(ex2v[:], arv[:, :, 1], 1.0 / NCORES)
            musq = sm.tile([P, 2], F32, name="musq2", tag="musq2")
            nc.vector.tensor_mul(musq[:], muv[:], muv[:])
            varv = sm.tile([P, 2], F32, name="varv", tag="varv")
            nc.vector.tensor_sub(varv[:], ex2v[:], musq[:])
            sigv = sm.tile([P, 2], F32, name="sigv", tag="sigv")
            nc.scalar.activation(sigv[:], varv[:], AF.Sqrt, bias=epsc[:])
            rgam = sm.tile([P, 2], F32, name="rgam", tag="rgam")
            nc.vector.reciprocal(rgam[:], gam2[:])
            tb = sm.tile([P, 2], F32, name="tb", tag="tb")
            nc.vector.tensor_mul(tb[:], bet2[:], sigv[:])
            tb2 = sm.tile([P, 2], F32, name="tb2", tag="tb2")
            nc.vector.tensor_mul(tb2[:], tb[:], rgam[:])
            tp = st.tile([P, 2], F32, name="tp", tag="tp")
            nc.vector.tensor_sub(tp[:], tb2[:], muv[:])

            # ---------------- pass 2: sign + conv + epilogue ----------------
            GRPS = [(0, 4), (4, 3)]  # (first chunk, n chunks) -> 4+3 banks

            def sign_img(img):
                for kc in range(2):
                    xqv = (xqb[:, img, kc, MARGIN:MARGIN + IMGP]
                           .rearrange("p (h w) -> p h w", w=WP))
                    nc.scalar.activation(
                        xqv[:, 1:1 + H, 1:1 + W],
                        xrv[:, kc, img, :].rearrange("p (h w) -> p h w", w=W),
                        AF.Sign, bias=tp[:, kc:kc + 1],
                    )

            def conv_img(img):
                for oc in range(2):
                    for gi, (c0, nch) in enumerate(GRPS):
                        pool = psA if gi == 0 else psB
                        tag = "cvA" if gi == 0 else "cvB"
                        cv = pool.tile([P, nch * BANK], F32, name="cv", tag=tag)
                        for tap in range(KTAPS):
                            dh, dw = tap // 3, tap % 3
                            off = (dh - 1) * WP + (dw - 1)
                            for ch in range(nch):
                                base = (MARGIN
                                        + (1 + (c0 + ch) * CH_ROWS) * WP + off)
                                nc.tensor.matmul(
                                    cv[:, ch * BANK:ch * BANK + CFP],
                                    wqv[:, tap, oc],
                                    xqb[:, img, :, base: base + CFP],
                                    start=(tap == 0), stop=(tap == KTAPS - 1),
                                    perf_mode=mybir.MatmulPerfMode.DoubleRow,
                                )
                        # fused relu(alpha*cv + alpha*b) over the whole group
                        cvv = (cv.rearrange("p (c x) -> p c x", x=BANK)
                               [:, :, 0:CFP]
                               .rearrange("p c (h w) -> p c h w", w=WP))
                        z = zp.tile([P, nch * CF], BF16, name="z", tag="z")
                        nc.scalar.activation(
                            z.rearrange("p (c h w) -> p c h w", c=nch, w=W),
                            cvv[:, :, :, 1:1 + W],
                            AF.Relu, bias=ab[oc][:], scale=alpha_sc[oc][:],
                        )
                        ot = outp.tile([P, nch * CF], BF16, name="ot", tag="ot")
                        nc.vector.tensor_mul(
                            ot[:], z[:],
                            abv[:, img, c0 * CF:(c0 + nch) * CF])
                        nc.sync.dma_start(
                            out_d.ap()[img, oc * P:(oc + 1) * P,
                                       c0 * CH_ROWS:(c0 + nch) * CH_ROWS, :],
                            ot.rearrange("p (r w) -> p r w", w=W),
                        )

            sign_img(0)
            for img in range(1, NL):
                sign_img(img)
                conv_img(img - 1)
            conv_img(NL - 1)

    nc.compile()
    return nc


_NC_CACHE: dict = {}


def _get_nc(n_local: int):
    if n_local not in _NC_CACHE:
        _NC_CACHE[n_local] = _build(n_local)
    return _NC_CACHE[n_local]


def _host_consts():
    ident = np.eye(P, dtype=np.float32).astype(NPBF16)
    tvt = np.zeros((HP, H), dtype=np.float32)
    for h in range(H):
        tvt[h:h + 3, h] = 1.0 / (9.0 * CIN)
    return ident, tvt.astype(NPBF16)


def _run(inputs: dict, trace: bool = False):
    x = np.asarray(inputs["x"], dtype=np.float32).astype(NPBF16)
    gamma = np.ascontiguousarray(np.asarray(inputs["gamma"], dtype=np.float32))
    beta_bn = np.ascontiguousarray(np.asarray(inputs["beta_bn"], dtype=np.float32))
    Wt = np.asarray(inputs["W"], dtype=np.float32).astype(NPBF16)
    b = np.ascontiguousarray(np.asarray(inputs["b"], dtype=np.float32))

    n = x.shape[0]
    assert n % NCORES == 0, f"batch {n} not divisible by {NCORES}"
    nl = n // NCORES
    nc = _get_nc(nl)
    ident, tvt = _host_consts()

    in_maps = []
    for i in range(NCORES):
        in_maps.append({
            "x": np.ascontiguousarray(x[i * nl:(i + 1) * nl]),
            "gamma": gamma, "beta_bn": beta_bn, "W": Wt, "b": b,
            "ident": ident, "tvt": tvt,
        })
    res = run_bass_kernel_spmd(nc, in_maps, core_ids=list(range(NCORES)),
                               trace=trace)
    out = np.concatenate(
        [res.results[i]["out"].astype(np.float32) for i in range(NCORES)],
        axis=0)
    return out, res


def kernel(**inputs) -> np.ndarray:
    out, _ = _run(inputs, trace=False)
    return out


def kernel_timed(**inputs):
    out, res = _run(inputs, trace=True)
    return out, res


# revision 19
# speedup vs baseline: 1.1981x; 1.1981x over previous
"""BinConv2d (XNOR-Net style) Trainium2 kernel, 8-core data-parallel, v3.

Layer math (BatchNorm train-mode -> BinActiv -> binary 3x3 conv -> scale by
box-filtered channel-mean magnitudes and per-filter alpha -> relu):

  mu, var: batch stats of x over (N, H, W) per channel      (needs all-reduce)
  xn  = (x - mu) * rsqrt(var + eps) * gamma + beta
  m   = mean_c |xn|;  xb = sign(xn);  Wb = sign(W);  alpha = mean |W| per filter
  y   = conv(xb, Wb, pad=1) + b
  out = relu(y * box3x3(m) * alpha)

Structure (the stats AllGather has ~70-120us ncfw setup latency; everything
that does not depend on the *global* stats runs while it is in flight):

  pass 1:  load x (bf16, host-converted) into resident SBUF, one bn_stats per
           image-half, trigger the AllGather as early as possible (~40us).
  window:  weight prep (PE transpose + ACT sign -> fp8, alpha via ACT
           abs-accumulate), and the m/beta path computed with LOCAL per-core
           BN stats (the m path is a smooth functional of the stats; local
           stats perturb it ~1e-3 relative, far under the harness gate):
           ACT abs -> fp8 |xn|, one DoubleRow matmul per row-chunk on a
           dedicated PSUM bank, DVE copies into the padded m plane, box3x3
           via DVE row-sums + banded matmul, then gpsimd partition_broadcast
           into abeta[128, img, pix] (bf16).
  pass 2:  global scalar chain -> t'; ACT sign (exact global stats) -> fp8
           conv via 9 shifted DoubleRow matmuls (PSUM 4+3 banks per image
           half, conv-exclusive) -> fused relu(scale,bias) -> multiply by
           abeta -> bf16 DMA out.

  sign needs *global* mu (a local-stats threshold flips ~0.24% of pixels =>
  ~10% output error), so the conv cannot start before the collective lands;
  the m path tolerates local stats.
"""

import os
import sys

import numpy as np

for _p in ("/opt/trn_rl_repo", "/root/.axon_site/_ro/trn_rl_repo"):
    if os.path.isdir(_p) and _p not in sys.path:
        sys.path.insert(0, _p)

import concourse.bass as bass  # noqa: E402
import concourse.bacc as bacc  # noqa: E402
import concourse.mybir as mybir  # noqa: E402
import concourse.tile as tile  # noqa: E402
from concourse.bass_utils import run_bass_kernel_spmd  # noqa: E402

F32 = mybir.dt.float32
BF16 = mybir.dt.bfloat16
FP8 = mybir.dt.float8e4
NPBF16 = mybir.dt.np(BF16)
AF = mybir.ActivationFunctionType
ALU = mybir.AluOpType
AX = mybir.AxisListType

EPS = 1e-4
NCORES = 8
P = 128
CIN = 256
COUT = 256
H = 56
W = 56
HP = H + 2          # 58 padded rows
WP = W + 2          # 58 padded cols
IMGP = HP * WP      # 3364 padded pixels / image
NPIX = H * W        # 3136 true pixels / image
MARGIN = 64         # dead zero margin absorbing out-of-image tap reads
CH_ROWS = 8         # output rows per PSUM bank
NCH = H // CH_ROWS  # 7 chunks / image
CF = CH_ROWS * W    # 448 compact free elems / chunk
CFP = CH_ROWS * WP  # 464 padded free elems / chunk
BANK = 512          # f32 elems per PSUM bank
KTAPS = 9
WARMUP = os.environ.get("BC_WARMUP", "1") == "1"


def _build(n_local: int):
    NL = n_local
    FREEPAD = 2 * MARGIN + NL * IMGP

    nc = bacc.Bacc("TRN2", debug=False, target_bir_lowering=False,
                   num_devices=NCORES)
    x_d = nc.declare_dram_parameter("x", [NL, CIN, H, W], BF16, isOutput=False)
    g_d = nc.declare_dram_parameter("gamma", [CIN], F32, isOutput=False)
    bb_d = nc.declare_dram_parameter("beta_bn", [CIN], F32, isOutput=False)
    w_d = nc.declare_dram_parameter("W", [COUT, CIN, 3, 3], BF16, isOutput=False)
    b_d = nc.declare_dram_parameter("b", [COUT], F32, isOutput=False)
    id_d = nc.declare_dram_parameter("ident", [P, P], BF16, isOutput=False)
    tv_d = nc.declare_dram_parameter("tvt", [HP, H], BF16, isOutput=False)
    out_d = nc.declare_dram_parameter("out", [NL, COUT, H, W], BF16, isOutput=True)

    with tile.TileContext(nc, num_cores=NCORES) as tc:
        with (
            tc.tile_pool(name="statics", bufs=1) as st,
            tc.tile_pool(name="wn", bufs=2) as wn_p,
            tc.tile_pool(name="axp", bufs=3) as axp,
            tc.tile_pool(name="smalls", bufs=2) as sm,
            tc.tile_pool(name="zp", bufs=2) as zp,
            tc.tile_pool(name="outp", bufs=3) as outp,
            tc.tile_pool(name="psA", bufs=1, space="PSUM") as psA,
            tc.tile_pool(name="psB", bufs=1, space="PSUM") as psB,
            tc.tile_pool(name="psS", bufs=1, space="PSUM") as psS,
            tc.tile_pool(name="dram", bufs=1, space="DRAM") as dr,
        ):
            # ---------------- warmup collective ----------------
            # ncfw pays ~70-120us of one-time setup on the first collective;
            # fire a throwaway AllGather with no data deps immediately so the
            # real one only pays mesh latency.
            if WARMUP:
                wu_in = dr.tile([1, 8], F32, name="wu_in", tag="wu_in")
                wu_out = dr.tile([NCORES, 1, 8], F32, name="wu_out",
                                 tag="wu_out", addr_space="Shared")
                nc.gpsimd.collective_compute(
                    "AllGather", ALU.bypass,
                    replica_groups=[list(range(NCORES))],
                    ins=[wu_in.opt()], outs=[wu_out.opt()],
                )

            # ---------------- pass 1: load x + BN statistics ----------------
            # (emitted first so the collective trigger lands ~40us in)
            xr = st.tile([P, 2 * NL * NPIX], BF16, name="xr", tag="xr")
            xrv = xr.rearrange("p (k i f) -> p k i f", k=2, i=NL)
            # DVE does bn_stats for kc0 (all imgs) + kc1 img0; ACT accumulates
            # raw sum/sumsq for kc1 imgs 1..3 (a pure-DVE pass-1 serializes 56
            # bn_stats and delays the collective trigger by ~25us).
            ACT_STATS = [(1, i) for i in range(1, NL)]
            stats = []
            for kc in range(2):
                ngrp = NCH * (NL if kc == 0 else 1)
                sb = st.tile([P, ngrp * 6], F32, name=f"stats{kc}",
                             tag=f"stats{kc}")
                stats.append(sb)
            acc_s, acc_q = {}, {}
            trash = st.tile([P, NPIX], BF16, name="trash", tag="trash")
            for img in range(NL):
                for kc in range(2):
                    deng = nc.sync if kc == 0 else nc.scalar
                    deng.dma_start(
                        xrv[:, kc, img, :],
                        x_d.ap()[img, kc * P:(kc + 1) * P]
                        .rearrange("c h w -> c (h w)"),
                    )
                    if (kc, img) in ACT_STATS:
                        s_t = sm.tile([P, 1], F32, name="s_t", tag=f"s_t{img}")
                        q_t = sm.tile([P, 1], F32, name="q_t", tag=f"q_t{img}")
                        nc.scalar.activation(trash[:], xrv[:, kc, img, :],
                                             AF.Copy, accum_out=s_t[:])
                        nc.scalar.activation(trash[:], xrv[:, kc, img, :],
                                             AF.Square, accum_out=q_t[:])
                        acc_s[(kc, img)] = s_t
                        acc_q[(kc, img)] = q_t
                    else:
                        ioff = img if kc == 0 else 0
                        for g in range(NCH):
                            col = (ioff * NCH + g) * 6
                            nc.vector.bn_stats(
                                stats[kc][:, col:col + 6],
                                xrv[:, kc, img, g * CF:(g + 1) * CF],
                            )
            # per-half (mean, ex2, var): kc0 pure bn_aggr; kc1 combines the
            # img0 aggregate with the ACT raw sums (equal image weights)
            NHALF = NL * NPIX
            mean_h, ex2_h, var_h = [], [], []
            pay = st.tile([P, 4], F32, name="pay", tag="pay")
            a0 = st.tile([P, 2], F32, name="a0", tag="a0")
            nc.vector.bn_aggr(a0[:], stats[0][:])
            m0sq = sm.tile([P, 1], F32, name="m0sq", tag="m0sq")
            nc.vector.tensor_mul(m0sq[:], a0[:, 0:1], a0[:, 0:1])
            ex0 = st.tile([P, 1], F32, name="ex0", tag="ex0")
            nc.vector.tensor_add(ex0[:], a0[:, 1:2], m0sq[:])
            nc.vector.tensor_copy(pay[:, 0:1], a0[:, 0:1])
            nc.vector.tensor_copy(pay[:, 1:2], ex0[:])
            mean_h.append(a0[:, 0:1]); ex2_h.append(ex0[:])
            var_h.append(a0[:, 1:2])
            a1 = st.tile([P, 2], F32, name="a1", tag="a1")
            nc.vector.bn_aggr(a1[:], stats[1][:])
            # S = NPIX*mean_img0 + sum(s_i); Q = NPIX*(var+mean^2) + sum(q_i)
            Ssum = sm.tile([P, 1], F32, name="Ssum", tag="Ssum")
            nc.vector.tensor_add(Ssum[:], acc_s[(1, 1)][:], acc_s[(1, 2)][:])
            nc.vector.tensor_add(Ssum[:], Ssum[:], acc_s[(1, 3)][:])
            m1n = sm.tile([P, 1], F32, name="m1n", tag="m1n")
            nc.vector.tensor_scalar_mul(m1n[:], a1[:, 0:1], float(NPIX))
            nc.vector.tensor_add(Ssum[:], Ssum[:], m1n[:])
            Qsum = sm.tile([P, 1], F32, name="Qsum", tag="Qsum")
            nc.vector.tensor_add(Qsum[:], acc_q[(1, 1)][:], acc_q[(1, 2)][:])
            nc.vector.tensor_add(Qsum[:], Qsum[:], acc_q[(1, 3)][:])
            m1sq = sm.tile([P, 1], F32, name="m1sq", tag="m1sq")
            nc.vector.tensor_mul(m1sq[:], a1[:, 0:1], a1[:, 0:1])
            e1 = sm.tile([P, 1], F32, name="e1", tag="e1")
            nc.vector.tensor_add(e1[:], a1[:, 1:2], m1sq[:])
            e1n = sm.tile([P, 1], F32, name="e1n", tag="e1n")
            nc.vector.tensor_scalar_mul(e1n[:], e1[:], float(NPIX))
            nc.vector.tensor_add(Qsum[:], Qsum[:], e1n[:])
            mean1 = st.tile([P, 1], F32, name="mean1", tag="mean1")
            nc.vector.tensor_scalar_mul(mean1[:], Ssum[:], 1.0 / NHALF)
            ex21 = st.tile([P, 1], F32, name="ex21", tag="ex21")
            nc.vector.tensor_scalar_mul(ex21[:], Qsum[:], 1.0 / NHALF)
            nc.vector.tensor_copy(pay[:, 2:3], mean1[:])
            nc.vector.tensor_copy(pay[:, 3:4], ex21[:])
            mn1sq = sm.tile([P, 1], F32, name="mn1sq", tag="mn1sq")
            nc.vector.tensor_mul(mn1sq[:], mean1[:], mean1[:])
            var1 = st.tile([P, 1], F32, name="var1", tag="var1")
            nc.vector.tensor_sub(var1[:], ex21[:], mn1sq[:])
            mean_h.append(mean1[:]); ex2_h.append(ex21[:])
            var_h.append(var1[:])
            cc_in = dr.tile([P, 4], F32, name="cc_in", tag="cc_in")
            cc_out = dr.tile([NCORES, P, 4], F32, name="cc_out", tag="cc_out",
                             addr_space="Shared")
            nc.sync.dma_start(cc_in[:], pay[:])
            nc.gpsimd.collective_compute(
                "AllGather", ALU.bypass,
                replica_groups=[list(range(NCORES))],
                ins=[cc_in.opt()], outs=[cc_out.opt()],
            )

            # ---------------- static pads (gpsimd; DVE stays on stats) ------
            # xq blocks: [P, img, ko, M | IMGP | M] so the DoubleRow rhs view
            # (k-stride = BLK) bounding-boxes only ONE image's two halves --
            # a [P, 2*plane] layout makes conv(img) falsely depend on later
            # images' sign writes (subtile deps use bounding ranges).
            BLK = IMGP + 2 * MARGIN
            xq = st.tile([P, NL * 2 * BLK], FP8, name="xq", tag="xq")
            xqb = xq.rearrange("p (i k b) -> p i k b", i=NL, k=2)
            for img in range(NL):
                for ko in range(2):
                    nc.gpsimd.memset(xqb[:, img, ko, 0:MARGIN + WP], 0.0)
                    nc.gpsimd.memset(
                        xqb[:, img, ko, MARGIN + (HP - 1) * WP:BLK], 0.0)
                    colv = (xqb[:, img, ko, MARGIN + WP: MARGIN + (HP - 1) * WP]
                            .rearrange("p (h w) -> p h w", w=WP))
                    nc.gpsimd.memset(colv[:, :, 0:1], 0.0)
                    nc.gpsimd.memset(colv[:, :, WP - 1:WP], 0.0)
            m_flat = st.tile([1, NL * IMGP], BF16, name="m_flat", tag="m_flat")
            mfl = m_flat.rearrange("p (i f) -> p i f", i=NL)
            nc.gpsimd.memset(mfl[:, :, 0:WP], 0.0)
            nc.gpsimd.memset(mfl[:, :, (HP - 1) * WP:IMGP], 0.0)
            mfv = mfl[:, :, WP:(HP - 1) * WP].rearrange("p i (h w) -> p i h w",
                                                        w=WP)
            nc.gpsimd.memset(mfv[:, :, :, 0:1], 0.0)
            nc.gpsimd.memset(mfv[:, :, :, WP - 1:WP], 0.0)
            epsc = st.tile([P, 1], F32, name="epsc", tag="epsc")
            nc.vector.memset(epsc[:], EPS)
            ones2 = st.tile([P, 2], FP8, name="ones2", tag="ones2")
            nc.vector.memset(ones2[:], 1.0)

            # ---------------- host constants ----------------
            ident = st.tile([P, P], BF16, name="ident_sb", tag="ident_sb")
            nc.scalar.dma_start(ident[:], id_d.ap())
            tvt = st.tile([HP, H], BF16, name="tvt_sb", tag="tvt_sb")
            nc.scalar.dma_start(tvt[:], tv_d.ap())
            gam2 = st.tile([P, 2], F32, name="gam2", tag="gam2")
            nc.scalar.dma_start(gam2[:], g_d.ap().rearrange("(k p) -> p k", k=2))
            bet2 = st.tile([P, 2], F32, name="bet2", tag="bet2")
            nc.scalar.dma_start(bet2[:], bb_d.ap().rearrange("(k p) -> p k", k=2))
            bvec2 = st.tile([P, 2], F32, name="bvec2", tag="bvec2")
            nc.scalar.dma_start(bvec2[:], b_d.ap().rearrange("(k p) -> p k", k=2))

            # ---------------- window: local-stat scalars ----------------
            s_loc, bstar = [], []
            for kc in range(2):
                sigl = sm.tile([P, 1], F32, name="sigl", tag="sigl")
                nc.scalar.activation(sigl[:], var_h[kc], AF.Sqrt,
                                     bias=epsc[:])
                rsigl = sm.tile([P, 1], F32, name="rsigl", tag="rsigl")
                nc.vector.reciprocal(rsigl[:], sigl[:])
                sl = st.tile([P, 1], F32, name=f"sl{kc}", tag=f"sl{kc}")
                nc.vector.tensor_mul(sl[:], gam2[:, kc:kc + 1], rsigl[:])
                s_loc.append(sl)
                smu = sm.tile([P, 1], F32, name="smu", tag="smu")
                nc.vector.tensor_mul(smu[:], sl[:], mean_h[kc])
                bs = st.tile([P, 1], F32, name=f"bs{kc}", tag=f"bs{kc}")
                nc.vector.tensor_sub(bs[:], bet2[:, kc:kc + 1], smu[:])
                bstar.append(bs)

            # ---------------- window: weight prep ----------------
            w_nat = []
            for oc in range(2):
                wn = wn_p.tile([P, CIN * KTAPS], BF16, name="w_nat", tag="wn")
                nc.sync.dma_start(
                    wn[:],
                    w_d.ap()[oc * P:(oc + 1) * P]
                    .rearrange("o c kh kw -> o (c kh kw)"),
                )
                w_nat.append(wn)
            # alpha via ACT |.| accumulate (tensor_reduce on DVE would race the
            # stats/pay chain and delay the collective trigger)
            alpha_sc, ab = [], []
            for oc in range(2):
                araw = sm.tile([P, 1], F32, name="araw", tag="araw")
                nc.scalar.activation(trash[:, 0:CIN * KTAPS], w_nat[oc][:],
                                     AF.Abs, accum_out=araw[:])
                asc = st.tile([P, 1], F32, name=f"alph{oc}", tag=f"alph{oc}")
                nc.vector.tensor_scalar_mul(asc[:], araw[:], 1.0 / (CIN * KTAPS))
                alpha_sc.append(asc)
                abt = st.tile([P, 1], F32, name=f"ab{oc}", tag=f"ab{oc}")
                nc.vector.tensor_mul(abt[:], asc[:], bvec2[:, oc:oc + 1])
                ab.append(abt)
            # wq: sign(W) transposed into DoubleRow lhsT layout
            # [P(ki), tap, oc, ko, m] with ko = channel half (c = ko*128+ki)
            wq = st.tile([P, KTAPS * 2 * 2 * P], FP8, name="wq", tag="wq")
            wqv = wq.rearrange("p (t o k m) -> p t o k m", t=KTAPS, o=2, k=2)
            for oc in range(2):
                wv = w_nat[oc][:].rearrange("o (c t) -> o c t", t=KTAPS)
                for kcw in range(2):
                    for tap in range(KTAPS):
                        pool, tag = ((psA, "cvA") if (tap + kcw) % 2 == 0
                                     else (psB, "cvB"))
                        psT = pool.tile([P, P], BF16, name="psT", tag=tag)
                        nc.tensor.transpose(
                            psT[:], wv[:, kcw * P:(kcw + 1) * P, tap], ident[:])
                        nc.scalar.activation(wqv[:, tap, oc, kcw, :], psT[:],
                                             AF.Sign)

            # ---------------- window: m path with LOCAL stats ----------------
            # ax = |s_loc*x + bstar_loc| = |xn_loc| in fp8 (3% elem error
            # averages to ~0.1% on m), one DoubleRow matmul per chunk against
            # all-ones [P,2,1], 1/(CIN*9) folded into tvt.
            abeta = st.tile([P, NL * NPIX], BF16, name="abeta", tag="abeta")
            abv = abeta.rearrange("p (i f) -> p i f", i=NL)
            for img in range(NL):
                ax = axp.tile([P, 2 * NPIX], FP8, name="ax", tag="ax")
                axv = ax.rearrange("p (k f) -> p k f", k=2)
                for kc in range(2):
                    nc.scalar.activation(axv[:, kc, :], xrv[:, kc, img, :],
                                         AF.Abs, bias=bstar[kc][:],
                                         scale=s_loc[kc][:])
                for ch in range(NCH):
                    mp = psS.tile([1, CF], F32, name="mp", tag="s")
                    nc.tensor.matmul(mp[:], ones2[:, 0:1],
                                     axv[:, 0, ch * CF:(ch + 1) * CF],
                                     start=True, stop=False)
                    nc.tensor.matmul(mp[:], ones2[:, 1:2],
                                     axv[:, 1, ch * CF:(ch + 1) * CF],
                                     start=False, stop=True)
                    mfi = (m_flat[:, img * IMGP:(img + 1) * IMGP]
                           .rearrange("p (h w) -> p h w", w=WP))
                    nc.vector.tensor_copy(
                        mfi[:, 1 + ch * CH_ROWS: 1 + (ch + 1) * CH_ROWS,
                            1:1 + W],
                        mp.rearrange("p (h w) -> p h w", w=W),
                    )
                # beta map: horizontal sum on DVE, vertical via banded matmul
                mhw = sm.tile([HP, WP], BF16, name="mhw", tag="mhw")
                nc.sync.dma_start(mhw[:], m_flat[:, img * IMGP:(img + 1) * IMGP])
                hs = sm.tile([HP, WP], BF16, name="hs", tag="hs")
                nc.vector.tensor_add(hs[:, 1:1 + W], mhw[:, 0:W], mhw[:, 2:2 + W])
                nc.vector.tensor_add(hs[:, 1:1 + W], hs[:, 1:1 + W],
                                     mhw[:, 1:1 + W])
                bps = psS.tile([H, W], F32, name="bps", tag="s")
                nc.tensor.matmul(bps[:], tvt[:], hs[:, 1:1 + W], start=True,
                                 stop=True)
                bhw = sm.tile([H, W], BF16, name="bhw", tag="bhw")
                nc.vector.tensor_copy(bhw[:], bps[:])
                bflat = sm.tile([1, NPIX], BF16, name="bflat", tag="bflat",
                                bufs=2)
                nc.sync.dma_start(bflat[:], bhw[:])
                nc.gpsimd.partition_broadcast(abv[:, img, :], bflat[:])

            # ---------------- global stats readback + scalars ----------------
            ag_sb = st.tile([P, NCORES * 4], F32, name="ag_sb", tag="ag_sb")
            nc.sync.dma_start(
                ag_sb[:].rearrange("p (r c) -> p r c", c=4),
                cc_out.rearrange("r p c -> p r c"),
            )
            arsb = st.tile([P, 4], F32, name="arsb", tag="arsb")
            nc.vector.tensor_reduce(
                arsb[:], ag_sb[:].rearrange("p (r c) -> p c r", c=4),
                axis=AX.X, op=ALU.add,
            )
            mue = st.tile([P, 4], F32, name="mue", tag="mue")
            nc.vector.tensor_scalar_mul(mue[:], arsb[:], 1.0 / NCORES)
            muev = mue.rearrange("p (c k) -> p c k", c=2)  # [P, kc, (mean,ex2)]
            muv = muev[:, :, 0]
            musq = sm.tile([P, 2], F32, name="musq2", tag="musq2")
            nc.vector.tensor_mul(musq[:], muv, muv)
            varv = sm.tile([P, 2], F32, name="varv", tag="varv")
            nc.vector.tensor_sub(varv[:], muev[:, :, 1], musq[:])
            # PE p-state warmup: a chain of tiny matmuls gated on the
            # collective result so the 2.4GHz ramp happens during the sign
            # lead-in instead of eating the first ~15us of the conv.
            wrm = sm.tile([P, 4], BF16, name="wrm", tag="wrm")
            nc.vector.tensor_copy(wrm[:], arsb[:])
            for _ in range(60):
                pd = psS.tile([P, 4], F32, name="pd", tag="s")
                nc.tensor.matmul(pd[:], ident[:], wrm[:], start=True, stop=True)
            sigv = sm.tile([P, 2], F32, name="sigv", tag="sigv")
            nc.scalar.activation(sigv[:], varv[:], AF.Sqrt, bias=epsc[:])
            rgam = sm.tile([P, 2], F32, name="rgam", tag="rgam")
            nc.vector.reciprocal(rgam[:], gam2[:])
            tb = sm.tile([P, 2], F32, name="tb", tag="tb")
            nc.vector.tensor_mul(tb[:], bet2[:], sigv[:])
            tb2 = sm.tile([P, 2], F32, name="tb2", tag="tb2")
            nc.vector.tensor_mul(tb2[:], tb[:], rgam[:])
            tp = st.tile([P, 2], F32, name="tp", tag="tp")
            nc.vector.tensor_sub(tp[:], tb2[:], muv)

            # ---------------- pass 2: sign + conv + epilogue ----------------
            GRPS = [(0, 4), (4, 3)]  # (first chunk, n chunks) -> 4+3 banks

            def sign_img(img):
                for kc in range(2):
                    xqv = (xqb[:, img, kc, MARGIN:MARGIN + IMGP]
                           .rearrange("p (h w) -> p h w", w=WP))
                    nc.scalar.activation(
                        xqv[:, 1:1 + H, 1:1 + W],
                        xrv[:, kc, img, :].rearrange("p (h w) -> p h w", w=W),
                        AF.Sign, bias=tp[:, kc:kc + 1],
                    )

            def conv_img(img):
                for oc in range(2):
                    for gi, (c0, nch) in enumerate(GRPS):
                        pool = psA if gi == 0 else psB
                        tag = "cvA" if gi == 0 else "cvB"
                        cv = pool.tile([P, nch * BANK], F32, name="cv", tag=tag)
                        for tap in range(KTAPS):
                            dh, dw = tap // 3, tap % 3
                            off = (dh - 1) * WP + (dw - 1)
                            for ch in range(nch):
                                base = (MARGIN
                                        + (1 + (c0 + ch) * CH_ROWS) * WP + off)
                                nc.tensor.matmul(
                                    cv[:, ch * BANK:ch * BANK + CFP],
                                    wqv[:, tap, oc],
                                    xqb[:, img, :, base: base + CFP],
                                    start=(tap == 0), stop=(tap == KTAPS - 1),
                                    perf_mode=mybir.MatmulPerfMode.DoubleRow,
                                )
                        # fused relu(alpha*cv + alpha*b) over the whole group
                        cvv = (cv.rearrange("p (c x) -> p c x", x=BANK)
                               [:, :, 0:CFP]
                               .rearrange("p c (h w) -> p c h w", w=WP))
                        z = zp.tile([P, nch * CF], BF16, name="z", tag="z")
                        nc.scalar.activation(
                            z.rearrange("p (c h w) -> p c h w", c=nch, w=W),
                            cvv[:, :, :, 1:1 + W],
                            AF.Relu, bias=ab[oc][:], scale=alpha_sc[oc][:],
                        )
                        ot = outp.tile([P, nch * CF], BF16, name="ot", tag="ot")
                        nc.vector.tensor_mul(
                            ot[:], z[:],
                            abv[:, img, c0 * CF:(c0 + nch) * CF])
                        nc.sync.dma_start(
                            out_d.ap()[img, oc * P:(oc + 1) * P,
                                       c0 * CH_ROWS:(c0 + nch) * CH_ROWS, :],
                            ot.rearrange("p (r w) -> p r w", w=W),
                        )

            sign_img(0)
            for img in range(1, NL):
                sign_img(img)
                conv_img(img - 1)
            conv_img(NL - 1)

    nc.compile()
    return nc


_NC_CACHE: dict = {}


def _get_nc(n_local: int):
    if n_local not in _NC_CACHE:
        _NC_CACHE[n_local] = _build(n_local)
    return _NC_CACHE[n_local]


def _host_consts():
    ident = np.eye(P, dtype=np.float32).astype(NPBF16)
    tvt = np.zeros((HP, H), dtype=np.float32)
    for h in range(H):
        tvt[h:h + 3, h] = 1.0 / (9.0 * CIN)
    return ident, tvt.astype(NPBF16)


def _run(inputs: dict, trace: bool = False):
    x = np.asarray(inputs["x"], dtype=np.float32).astype(NPBF16)
    gamma = np.ascontiguousarray(np.asarray(inputs["gamma"], dtype=np.float32))
    beta_bn = np.ascontiguousarray(np.asarray(inputs["beta_bn"], dtype=np.float32))
    Wt = np.asarray(inputs["W"], dtype=np.float32).astype(NPBF16)
    b = np.ascontiguousarray(np.asarray(inputs["b"], dtype=np.float32))

    n = x.shape[0]
    assert n % NCORES == 0, f"batch {n} not divisible by {NCORES}"
    nl = n // NCORES
    nc = _get_nc(nl)
    ident, tvt = _host_consts()

    in_maps = []
    for i in range(NCORES):
        in_maps.append({
            "x": np.ascontiguousarray(x[i * nl:(i + 1) * nl]),
            "gamma": gamma, "beta_bn": beta_bn, "W": Wt, "b": b,
            "ident": ident, "tvt": tvt,
        })
    res = run_bass_kernel_spmd(nc, in_maps, core_ids=list(range(NCORES)),
                               trace=trace)
    out = np.concatenate(
        [res.results[i]["out"].astype(np.float32) for i in range(NCORES)],
        axis=0)
    return out, res


def kernel(**inputs) -> np.ndarray:
    out, _ = _run(inputs, trace=False)
    return out


def kernel_timed(**inputs):
    out, res = _run(inputs, trace=True)
    return out, res
